# revision 47
# baseline (speedup 1.0000x reference)
"""Fastmax (p=1 causal linear attention) Trainium2 kernel, 8-core SPMD.

Sharding: data-parallel over heads (16 heads -> 2 per core). Each core
computes q/k/v projections for its 2 heads, the chunked causal linear
attention (a [65,65] prefix state per head carries S, ksum, vsum, count),
and a partial output projection. Host sums the 8 partial outputs + bias.

v3 design notes (cost-model driven):
  - xt streamed as big [128, N] DMAs; phase A (q,k) accumulates k-outer
    into 8 resident psum banks so PE consumption paces DMA arrival. PE
    warmup matmuls off a memset tile beat the p-state ramp.
  - q/k stay HEAD-STACKED [128, 2N] in SBUF: one [128,512] drain copy
    per projection psum, so psum banks recycle ~650ns after stop and
    the drain never serializes pass 2.
  - normalization: score' = <q,k> + qn*kn is a uniform 1/s scaling of
    the reference score, so num/den is invariant; the +c term enters
    scores via a rank-1 matmul with a tiny c-filled row, and enters
    the O accumulation via the state's aug row.
  - norm stats: squares of the stacked copies (Pool/DVE), 8 stationary-
    one-hot matmuls stack into one [16,512] psum, single DVE max-reduce,
    PE transpose, tiny strided reduces.
  - krows (token-major k for the state sweep) come from PE transposes
    of the stacked k block; aug columns are memset ones.
  - causal state is chunk-granular (128 tokens): per chunk
    O = masked-diag + q @ S_prev + 1 * S_prev[aug], no unmasked block.
"""

import sys

sys.path.insert(0, "/opt/trn_rl_repo")

import numpy as np

B, N, D_MODEL, H, D_HEAD = 1, 2048, 1024, 16, 64
NCORES = 8
HPC = H // NCORES  # heads per core
DPC = HPC * D_HEAD  # out dims per core (128)
CH = 128  # chunk (tokens)
SPAN = 256  # query span (2 chunks)
NSPAN = N // SPAN
KT = D_MODEL // 128  # contraction tiles for projections
NCH = N // CH  # token chunks (16)
RST = 80  # row-buffer stride per chunk (64 data + ones col + pad)
HRST = NCH * RST  # per-head stride inside krows/vrows
NWARM = 6  # PE warmup matmuls during initial DMA wait

_CACHE = {}


def _build():
    import concourse.bass as bass
    import concourse.tile as tile
    import concourse.mybir as mybir
    from concourse import bacc
    from concourse.alu_op_type import AluOpType

    BF = mybir.dt.bfloat16
    F32 = mybir.dt.float32
    AF = mybir.ActivationFunctionType
    AX = mybir.AxisListType

    nc = bacc.Bacc("TRN2", target_bir_lowering=False, debug=False, num_devices=NCORES)

    xt_d = nc.declare_dram_parameter("xt", [D_MODEL, N], BF, isOutput=False)
    wqk_d = nc.declare_dram_parameter("wqk", [128, 2 * D_MODEL], BF, isOutput=False)
    wvo_d = nc.declare_dram_parameter("wvo", [128, 2 * D_MODEL], BF, isOutput=False)
    consts_d = nc.declare_dram_parameter("consts", [128, 896], BF, isOutput=False)
    out_d = nc.declare_dram_parameter("out", [N, D_MODEL], BF, isOutput=True)

    def ap3(t, off, free_dims, nparts=128):
        # raw AP: partition dim step is the tile's row pitch (elements)
        pitch = t.ap[0][0]
        return bass.AP(t.tensor, t.offset + off, [[pitch, nparts]] + free_dims)

    with tile.TileContext(nc) as tc:
        with (
            tc.tile_pool(name="const", bufs=1) as constp,
            tc.tile_pool(name="wts", bufs=1) as wp,
            tc.tile_pool(name="acts", bufs=1) as actp,
            tc.tile_pool(name="mt", bufs=4) as mtp,
            tc.tile_pool(name="ssb", bufs=1) as ssbp,
            tc.tile_pool(name="recp", bufs=8) as recp,
            tc.tile_pool(name="vhrp", bufs=4) as vhrp,
            tc.tile_pool(name="osb", bufs=3) as osbp,
            tc.tile_pool(name="nf", bufs=1) as nf,
        ):
            consts = constp.tile([128, 896], BF)
            ident = consts[:, 0:128]
            dmask4 = consts[:, 128:640]
            onesall = consts[:, 768:896]

            # head-stacked q|k activations: rows h*64..h*64+63 = head h dims,
            # columns [q (N) | k (N)]
            qks = actp.tile([128, HPC * N], BF, tag="qks")

            wqk_sb = wp.tile([128, 2 * D_MODEL], BF)
            nc.sync.dma_start(wqk_sb[:, 0:D_MODEL], wqk_d[:, 0:D_MODEL])
            nc.sync.dma_start(wqk_sb[:, D_MODEL : 2 * D_MODEL], wqk_d[:, D_MODEL : 2 * D_MODEL])

            xts = []
            for k in range(KT):
                xtile = actp.tile([128, N], BF, tag=f"xt{k}", name=f"xt{k}")
                if k < 2:  # halves: smoother early PE pacing
                    nc.sync.dma_start(xtile[:, 0 : N // 2], xt_d[k * 128 : (k + 1) * 128, 0 : N // 2])
                    nc.sync.dma_start(xtile[:, N // 2 : N], xt_d[k * 128 : (k + 1) * 128, N // 2 : N])
                else:
                    nc.sync.dma_start(xtile[:], xt_d[k * 128 : (k + 1) * 128, :])
                if k == 1:  # consts not needed before pass 2
                    nc.sync.dma_start(consts[:], consts_d[:])
                xts.append(xtile)

            wvo_sb = wp.tile([128, 2 * D_MODEL], BF)
            nc.sync.dma_start(wvo_sb[:], wvo_d[:])

            vht = actp.tile([128, N], BF, tag="vht")
            krows = actp.tile([128, HPC * HRST], BF, tag="krows")
            vrows = actp.tile([128, HPC * HRST], BF, tag="vrows")
            # warmup source first in the Pool queue: the PE ramp can begin
            # ~0.5us in, before any DMA lands
            wsrc = actp.tile([128, 512], BF, tag="wsrc")
            nc.gpsimd.memset(wsrc[:], 0.0)
            # dummy sqrt pins the act table that holds Copy+Sqrt, so the
            # c-path later needs no table reload
            sqtrash = actp.tile([1, 1], F32, tag="sqtrash")
            nc.scalar.activation(sqtrash[:], wsrc[0:1, 0:1], AF.Sqrt)
            # aug columns (64 mod RST) must be ones; data cols are
            # overwritten by the v / k-transpose copies later
            nc.gpsimd.memset(vrows[:], 1.0)
            nc.gpsimd.memset(krows[:], 1.0)
            # per-head c rows for the rank-1 score aug term (both at
            # partition 0; scaled in place once the norms resolve)
            crow = [
                actp.tile([1, 128], BF, tag=f"crow{h}", name=f"crow{h}") for h in range(HPC)
            ]
            for h in range(HPC):
                nc.gpsimd.memset(crow[h][:], 1.0)

            def qcol(h, c0, c1):  # stacked q slice for head h
                return qks[h * 64 : (h + 1) * 64, c0:c1]

            def kcol(h, c0, c1):
                return qks[h * 64 : (h + 1) * 64, N + c0 : N + c1]

            sqp = {}

            # ================= warmup + phase A: q,k projections =================
            with tc.tile_pool(name="warm", bufs=1, space="PSUM") as warmp:
                wps = warmp.tile([128, 512], F32)
                for i in range(NWARM):
                    nc.tensor.matmul(wps[:], wsrc[:, 0:128], wsrc[:], start=True, stop=True)

            with tc.tile_pool(name="projps", bufs=1, space="PSUM") as pps:
                pq = [pps.tile([128, 512], F32, tag=f"pq{n0}", name=f"pq{n0}") for n0 in range(4)]
                pk = [pps.tile([128, 512], F32, tag=f"pk{n0}", name=f"pk{n0}") for n0 in range(3)]

                def drain(p, name, isq, n0):
                    # single stacked copy: the psum bank frees after one op
                    cs0 = (0 if isq else N) + n0 * 512
                    dst = qks[:, cs0 : cs0 + 512]
                    if drain.cnt % 2 == 0:
                        nc.vector.tensor_copy(dst, p[:])
                    else:
                        nc.scalar.copy(dst, p[:])
                    drain.cnt += 1
                    # squares for the norm stats, off the SBUF copy
                    sq = actp.tile([128, 512], BF, tag=f"sq{name}", name=f"sq{name}")
                    if drain.cnt % 2 == 0:
                        nc.gpsimd.tensor_mul(sq[:], dst, dst)
                    else:
                        nc.vector.tensor_mul(sq[:], dst, dst)
                    sqp[name] = sq

                drain.cnt = 0

                for k in range(KT):
                    ws = wqk_sb[:, k * 128 : (k + 1) * 128]
                    wsk = wqk_sb[:, D_MODEL + k * 128 : D_MODEL + (k + 1) * 128]
                    for n0 in range(4):
                        nc.tensor.matmul(
                            pq[n0][:],
                            ws,
                            xts[k][:, n0 * 512 : (n0 + 1) * 512],
                            start=(k == 0),
                            stop=(k == KT - 1),
                        )
                    if k == KT - 1:
                        for n0 in range(4):
                            drain(pq[n0], f"q{n0}", True, n0)
                    for n0 in range(3):
                        nc.tensor.matmul(
                            pk[n0][:],
                            wsk,
                            xts[k][:, n0 * 512 : (n0 + 1) * 512],
                            start=(k == 0),
                            stop=(k == KT - 1),
                        )
                    if k == KT - 1:
                        for n0 in range(3):
                            drain(pk[n0], f"k{n0}", False, n0)

                # deferred k n0=3 rotates into the freed warmup bank
                pk3 = pps.tile([128, 512], F32, tag="pq0", name="pk3")
                for k in range(KT):
                    nc.tensor.matmul(
                        pk3[:],
                        wqk_sb[:, D_MODEL + k * 128 : D_MODEL + (k + 1) * 128],
                        xts[k][:, 3 * 512 : 4 * 512],
                        start=(k == 0),
                        stop=(k == KT - 1),
                    )
                drain(pk3, "k3", False, 3)

            # ============ region 2: norms, v, k-transposes, attention ============
            with (
                tc.tile_pool(name="vkps", bufs=2, space="PSUM") as vkps,
                tc.tile_pool(name="bigps", bufs=2, space="PSUM") as bigps,
            ):
                nr16 = nf.tile([16, 1], BF)

                s_chain = {}
                s_snap = {}  # (h, ci) -> ([64,65] at base h*64, [1,65] aug row)
                oops_h = {}

                def vktr_chunk(ci):
                    ts0 = ci * 128
                    pv = vkps.tile([128, 128], F32, tag="vk", name="pv")
                    for k in range(KT):
                        nc.tensor.matmul(
                            pv[:],
                            xts[k][:, ts0 : ts0 + 128],
                            wvo_sb[:, k * 128 : (k + 1) * 128],
                            start=(k == 0),
                            stop=(k == KT - 1),
                        )
                    dst = ap3(vrows[:], ci * RST, [[HRST, HPC], [1, 64]])
                    src = ap3(pv[:], 0, [[64, HPC], [1, 64]])
                    nc.scalar.copy(dst, src)
                    ktp = vkps.tile([128, 128], BF, tag="vk", name="ktp")
                    nc.tensor.transpose(
                        ktp[:], qks[:, N + ts0 : N + ts0 + 128], ident
                    )
                    dst = ap3(krows[:], ci * RST, [[HRST, HPC], [1, 64]])
                    src = ap3(ktp[:], 0, [[64, HPC], [1, 64]])
                    nc.scalar.copy(dst, src)

                def sweep_chunk(ci):
                    dl = vkps.tile([65, 2 * 65], F32, tag="vk", name="dl")
                    for h in range(HPC):
                        nc.tensor.matmul(
                            dl[:, h * 65 : (h + 1) * 65],
                            krows[:, h * HRST + ci * RST : h * HRST + ci * RST + 65],
                            vrows[:, h * HRST + ci * RST : h * HRST + ci * RST + 65],
                            start=True,
                            stop=True,
                            skip_group_check=True,
                        )
                    ch = ssbp.tile(
                        [65, 2 * 65], F32, tag=f"sch{ci}", name=f"sch{ci}", bufs=1
                    )
                    if ci == 0:
                        nc.vector.tensor_copy(ch[:], dl[:])
                    else:
                        nc.vector.tensor_add(ch[:], dl[:], s_chain[ci - 1][:])
                    s_chain[ci] = ch
                    for h in range(HPC):
                        sm = ssbp.tile(
                            [128, 65], BF, tag=f"ssb{h}_{ci}", name=f"ssb{h}_{ci}", bufs=1
                        )
                        nc.gpsimd.tensor_copy(
                            sm[h * 64 : (h + 1) * 64, :], ch[0:64, h * 65 : (h + 1) * 65]
                        )
                        sa = ssbp.tile(
                            [1, 65], BF, tag=f"ssa{h}_{ci}", name=f"ssa{h}_{ci}", bufs=1
                        )
                        nc.gpsimd.tensor_copy(sa[:], ch[64:65, h * 65 : (h + 1) * 65])
                        s_snap[(h, ci)] = (sm, sa)

                def nrm_mms(nrm16):
                    for j in range(8):
                        name = f"q{j}" if j < 4 else f"k{j - 4}"
                        nc.tensor.matmul(
                            nrm16[:],
                            consts[:, 640 + 16 * j : 656 + 16 * j],
                            sqp[name][:],
                            start=(j == 0),
                            stop=(j == 7),
                            skip_group_check=True,
                        )

                def c_finalize(tr16):
                    # tr16 column j holds mm j's (h0,h1) maxima pair; q cols
                    # {h,2+h,..}, k cols {8+h,..}. tiny strided reduces -> c_h.
                    for h in range(HPC):
                        mqh = nf.tile([1, 1], F32, tag=f"mq{h}", name=f"mq{h}")
                        mkh = nf.tile([1, 1], F32, tag=f"mk{h}", name=f"mk{h}")
                        nc.vector.tensor_reduce(
                            mqh[:], ap3(tr16[:], h, [[2, 4]], nparts=1), AX.X, AluOpType.max
                        )
                        nc.vector.tensor_reduce(
                            mkh[:], ap3(tr16[:], 8 + h, [[2, 4]], nparts=1), AX.X, AluOpType.max
                        )
                        pr = nf.tile([1, 1], F32, tag=f"pr{h}", name=f"pr{h}")
                        nc.vector.tensor_mul(pr[:], mqh[:], mkh[:])
                        rt = nf.tile([1, 1], F32, tag=f"rt{h}", name=f"rt{h}")
                        nc.scalar.activation(rt[:], pr[:], AF.Sqrt)
                        nc.vector.tensor_scalar_mul(crow[h][:], crow[h][:], rt[:])

                def scores(sp):
                    ptj = bigps.tile([128, 2 * SPAN], F32, tag="big", name="ptj")
                    for h in range(HPC):
                        for i, ci in enumerate((2 * sp, 2 * sp + 1)):
                            cs = ptj[:, (2 * h + i) * CH : (2 * h + i + 1) * CH]
                            nc.tensor.matmul(
                                cs,
                                kcol(h, ci * CH, (ci + 1) * CH),
                                qcol(h, ci * CH, (ci + 1) * CH),
                                start=True,
                                stop=False,
                                skip_group_check=True,
                            )
                            nc.tensor.matmul(
                                cs,
                                crow[h][:],
                                onesall[0:1, :],
                                start=False,
                                stop=True,
                                skip_group_check=True,
                            )
                    mtd = mtp.tile([128, 2 * SPAN], BF, tag="mtd", name="mtd")
                    nc.vector.tensor_mul(mtd[:], ptj[:], dmask4)
                    return mtd

                def o_part(sp, mtd):
                    ca, cb_ = 2 * sp, 2 * sp + 1
                    vhrs = {
                        ca: vhrp.tile([128, 128], BF, tag="vhr", name="vhra"),
                        cb_: vhrp.tile([128, 128], BF, tag="vhr", name="vhrb"),
                    }
                    for h in range(HPC):
                        for i, ci in enumerate((ca, cb_)):
                            o = oops_h["p"].tile([128, 65], F32, tag="oop", name="o")
                            vr = vrows[:, h * HRST + ci * RST : h * HRST + ci * RST + 65]
                            nc.tensor.matmul(
                                o[:],
                                mtd[:, (2 * h + i) * CH : (2 * h + i + 1) * CH],
                                vr,
                                start=True,
                                stop=(ci == 0),
                            )
                            if ci > 0:
                                sm, sa = s_snap[(h, ci - 1)]
                                nc.tensor.matmul(
                                    o[:],
                                    qcol(h, ci * CH, (ci + 1) * CH),
                                    sm[h * 64 : (h + 1) * 64, :],
                                    start=False,
                                    stop=False,
                                )
                                nc.tensor.matmul(
                                    o[:],
                                    crow[h][:],
                                    sa[:],
                                    start=False,
                                    stop=True,
                                )
                            rec = recp.tile([128, 1], F32, tag="rec", name="rec")
                            nc.vector.reciprocal(rec[:], o[:, 64:65])
                            dst = vhrs[ci][:, h * 64 : (h + 1) * 64]
                            if h == 1 and i == 1:
                                nc.scalar.activation(dst, o[:, 0:64], AF.Copy, scale=rec[:])
                            else:
                                nc.vector.tensor_scalar_mul(dst, o[:, 0:64], rec[:])
                    return vhrs

                def vht_finish(sp, vhrs):
                    vtp = bigps.tile([128, SPAN], BF, tag="big", name="vtp")
                    for i, ci in enumerate((2 * sp, 2 * sp + 1)):
                        nc.tensor.transpose(vtp[:, i * CH : (i + 1) * CH], vhrs[ci][:], ident)
                        dst = vht[:, ci * CH : (ci + 1) * CH]
                        if i == 0:
                            nc.scalar.copy(dst, vtp[:, i * CH : (i + 1) * CH])
                        else:
                            nc.vector.tensor_copy(dst, vtp[:, i * CH : (i + 1) * CH])

                def outproj_row(r):
                    if True:
                        rs_ = slice(r * CH, (r + 1) * CH)
                        ob = osbp.tile([128, D_MODEL], BF, tag="osb", name="osb")
                        for n2 in range(D_MODEL // 512):
                            ns = slice(n2 * 512, (n2 + 1) * 512)
                            op = oops_h["p"].tile([128, 512], F32, tag="oop", name="opps")
                            nc.tensor.matmul(
                                op[:],
                                vht[:, rs_],
                                wvo_sb[:, D_MODEL + ns.start : D_MODEL + ns.stop],
                                start=True,
                                stop=True,
                            )
                            if (r + n2) % 2 == 0:
                                nc.vector.tensor_copy(ob[:, ns], op[:])
                            else:
                                nc.scalar.copy(ob[:, ns], op[:])
                        nc.sync.dma_start(out_d[rs_, :], ob[:])

                nxt = [0]

                def emit_chunks_until(limit):
                    while nxt[0] <= min(limit, NCH - 1):
                        ci = nxt[0]
                        vktr_chunk(ci)
                        if ci < NCH - 1:
                            sweep_chunk(ci)
                        nxt[0] += 1

                # prelude: chunks 0..3 cover the PE while the norm chain and
                # c resolve on the side engines; nrmps lives only here so its
                # bank can go to the 3-deep O/outproj pool afterwards
                with tc.tile_pool(name="nrmps", bufs=1, space="PSUM") as nps:
                    nrm16 = nps.tile([16, 512], F32, tag="nrm", name="nrm16")
                    tr16 = nps.tile([1, 16], BF, tag="nrm", name="tr16")
                    emit_chunks_until(3)
                    nrm_mms(nrm16)
                    nc.vector.tensor_reduce(nr16[:], nrm16[:], AX.X, AluOpType.max)
                    emit_chunks_until(5)
                    nc.tensor.transpose(tr16[:], nr16[:], ident[0:16, 0:16])
                    c_finalize(tr16)

                with tc.tile_pool(name="oops", bufs=4, space="PSUM") as oops_pool:
                    oops_h["p"] = oops_pool
                    vhr_prev = None
                    for sp in range(NSPAN):
                        emit_chunks_until(2 * sp + 2)
                        if sp >= 1:
                            vht_finish(sp - 1, vhr_prev)
                        emit_chunks_until(2 * sp + 3)
                        mtd = scores(sp)
                        if sp >= 1:
                            outproj_row(2 * sp - 2)
                            outproj_row(2 * sp - 1)
                        vhr_prev = o_part(sp, mtd)
                    vht_finish(NSPAN - 1, vhr_prev)
                    outproj_row(NCH - 2)
                    outproj_row(NCH - 1)

    nc.compile()
    return nc


def _consts():
    import ml_dtypes

    bf = ml_dtypes.bfloat16
    consts = np.zeros((128, 896), dtype=np.float32)
    consts[:, 0:128] = np.eye(128)
    j = np.arange(128)[:, None]
    i = np.arange(CH)[None, :]
    tri = (j <= i).astype(np.float32)
    for b in range(4):
        consts[:, 128 + b * CH : 128 + (b + 1) * CH] = tri
    # hindt16 blocks: mm j's stationary [128,16] has only cols 2j (head0
    # rows) and 2j+1 (head1 rows) set, so 8 accumulating matmuls stack
    # per-(proj,n0) norm rows into one [16,512] psum.
    for jj in range(8):
        for h in range(HPC):
            consts[h * 64 : (h + 1) * 64, 640 + 16 * jj + 2 * jj + h] = 1.0
    consts[:, 768:896] = 1.0  # onesall
    return consts.astype(bf)


def _in_maps(inputs):
    import ml_dtypes

    bf = ml_dtypes.bfloat16
    X = np.ascontiguousarray(np.asarray(inputs["X"], dtype=np.float32))
    xt = np.ascontiguousarray(X[0].T).astype(bf)  # [D_MODEL, N]
    wqt = np.ascontiguousarray(np.asarray(inputs["Wq"], np.float32).T).astype(bf)
    wkt = np.ascontiguousarray(np.asarray(inputs["Wk"], np.float32).T).astype(bf)
    wvt = np.ascontiguousarray(np.asarray(inputs["Wv"], np.float32).T).astype(bf)
    wot = np.ascontiguousarray(np.asarray(inputs["Wo"], np.float32).T).astype(bf)
    consts = _consts()

    def sb_layout(w):  # [1024, 128] -> [128, 8*128] (dm-chunk on partitions)
        return np.ascontiguousarray(
            w.reshape(KT, 128, DPC).transpose(1, 0, 2).reshape(128, KT * DPC)
        )

    in_maps = []
    for c in range(NCORES):
        cs = slice(c * DPC, (c + 1) * DPC)
        wqk = np.concatenate([sb_layout(wqt[:, cs]), sb_layout(wkt[:, cs])], axis=1)
        wvo = np.concatenate(
            [sb_layout(wvt[:, cs]), np.ascontiguousarray(wot[cs, :])], axis=1
        )
        in_maps.append(
            {
                "xt": xt,
                "wqk": np.ascontiguousarray(wqk),
                "wvo": np.ascontiguousarray(wvo),
                "consts": consts,
            }
        )
    return in_maps


def _run(inputs, trace=False):
    from concourse.bass_utils import run_bass_kernel_spmd

    if "nc" not in _CACHE:
        _CACHE["nc"] = _build()
    nc = _CACHE["nc"]
    in_maps = _in_maps(inputs)
    res = run_bass_kernel_spmd(nc, in_maps, core_ids=list(range(NCORES)), trace=trace)
    bo = np.asarray(inputs["bo"], dtype=np.float32)
    acc = np.zeros((N, D_MODEL), dtype=np.float32)
    for c in range(NCORES):
        acc += res.results[c]["out"].astype(np.float32)
    acc += bo[None, :]
    return acc.reshape(B, N, D_MODEL), res.exec_time_ns


def kernel(**inputs) -> np.ndarray:
    out, _ = _run(inputs, trace=False)
    return out


# revision 48
# speedup vs baseline: 1.0059x; 1.0059x over previous
"""Fastmax (p=1 causal linear attention) Trainium2 kernel, 8-core SPMD.

Sharding: data-parallel over heads (16 heads -> 2 per core). Each core
computes q/k/v projections for its 2 heads, the chunked causal linear
attention (a [65,65] prefix state per head carries S, ksum, vsum, count),
and a partial output projection. Host sums the 8 partial outputs + bias.

v3 design notes (cost-model driven):
  - xt streamed as big [128, N] DMAs; phase A (q,k) accumulates k-outer
    into 8 resident psum banks so PE consumption paces DMA arrival. PE
    warmup matmuls off a memset tile beat the p-state ramp.
  - q/k stay HEAD-STACKED [128, 2N] in SBUF: one [128,512] drain copy
    per projection psum, so psum banks recycle ~650ns after stop and
    the drain never serializes pass 2.
  - normalization: score' = <q,k> + qn*kn is a uniform 1/s scaling of
    the reference score, so num/den is invariant; the +c term enters
    scores via a rank-1 matmul with a tiny c-filled row, and enters
    the O accumulation via the state's aug row.
  - norm stats: squares of the stacked copies (Pool/DVE), 8 stationary-
    one-hot matmuls stack into one [16,512] psum, single DVE max-reduce,
    PE transpose, tiny strided reduces.
  - krows (token-major k for the state sweep) come from PE transposes
    of the stacked k block; aug columns are memset ones.
  - causal state is chunk-granular (128 tokens): per chunk
    O = masked-diag + q @ S_prev + 1 * S_prev[aug], no unmasked block.
"""

import sys

sys.path.insert(0, "/opt/trn_rl_repo")

import numpy as np

B, N, D_MODEL, H, D_HEAD = 1, 2048, 1024, 16, 64
NCORES = 8
HPC = H // NCORES  # heads per core
DPC = HPC * D_HEAD  # out dims per core (128)
CH = 128  # chunk (tokens)
SPAN = 256  # query span (2 chunks)
NSPAN = N // SPAN
KT = D_MODEL // 128  # contraction tiles for projections
NCH = N // CH  # token chunks (16)
RST = 80  # row-buffer stride per chunk (64 data + ones col + pad)
HRST = NCH * RST  # per-head stride inside krows/vrows
NWARM = 6  # PE warmup matmuls during initial DMA wait

_CACHE = {}


def _build():
    import concourse.bass as bass
    import concourse.tile as tile
    import concourse.mybir as mybir
    from concourse import bacc
    from concourse.alu_op_type import AluOpType

    BF = mybir.dt.bfloat16
    F32 = mybir.dt.float32
    AF = mybir.ActivationFunctionType
    AX = mybir.AxisListType

    nc = bacc.Bacc("TRN2", target_bir_lowering=False, debug=False, num_devices=NCORES)

    xt_d = nc.declare_dram_parameter("xt", [D_MODEL, N], BF, isOutput=False)
    wqk_d = nc.declare_dram_parameter("wqk", [128, 2 * D_MODEL], BF, isOutput=False)
    wvo_d = nc.declare_dram_parameter("wvo", [128, 2 * D_MODEL], BF, isOutput=False)
    consts_d = nc.declare_dram_parameter("consts", [128, 896], BF, isOutput=False)
    out_d = nc.declare_dram_parameter("out", [N, D_MODEL], BF, isOutput=True)

    def ap3(t, off, free_dims, nparts=128):
        # raw AP: partition dim step is the tile's row pitch (elements)
        pitch = t.ap[0][0]
        return bass.AP(t.tensor, t.offset + off, [[pitch, nparts]] + free_dims)

    with tile.TileContext(nc) as tc:
        with (
            tc.tile_pool(name="const", bufs=1) as constp,
            tc.tile_pool(name="wts", bufs=1) as wp,
            tc.tile_pool(name="acts", bufs=1) as actp,
            tc.tile_pool(name="mt", bufs=4) as mtp,
            tc.tile_pool(name="ssb", bufs=1) as ssbp,
            tc.tile_pool(name="recp", bufs=8) as recp,
            tc.tile_pool(name="vhrp", bufs=4) as vhrp,
            tc.tile_pool(name="osb", bufs=3) as osbp,
            tc.tile_pool(name="nf", bufs=1) as nf,
        ):
            consts = constp.tile([128, 896], BF)
            ident = consts[:, 0:128]
            dmask4 = consts[:, 128:640]
            onesall = consts[:, 768:896]

            # head-stacked q|k activations: rows h*64..h*64+63 = head h dims,
            # columns [q (N) | k (N)]
            qks = actp.tile([128, HPC * N], BF, tag="qks")

            wqk_sb = wp.tile([128, 2 * D_MODEL], BF)
            nc.sync.dma_start(wqk_sb[:, 0:D_MODEL], wqk_d[:, 0:D_MODEL])
            nc.sync.dma_start(wqk_sb[:, D_MODEL : 2 * D_MODEL], wqk_d[:, D_MODEL : 2 * D_MODEL])

            xts = []
            for k in range(KT):
                xtile = actp.tile([128, N], BF, tag=f"xt{k}", name=f"xt{k}")
                if k < 2:  # halves: smoother early PE pacing
                    nc.sync.dma_start(xtile[:, 0 : N // 2], xt_d[k * 128 : (k + 1) * 128, 0 : N // 2])
                    nc.sync.dma_start(xtile[:, N // 2 : N], xt_d[k * 128 : (k + 1) * 128, N // 2 : N])
                else:
                    nc.sync.dma_start(xtile[:], xt_d[k * 128 : (k + 1) * 128, :])
                if k == 1:  # consts not needed before pass 2
                    nc.sync.dma_start(consts[:], consts_d[:])
                xts.append(xtile)

            wvo_sb = wp.tile([128, 2 * D_MODEL], BF)
            nc.sync.dma_start(wvo_sb[:], wvo_d[:])

            vht = actp.tile([128, N], BF, tag="vht")
            krows = actp.tile([128, HPC * HRST], BF, tag="krows")
            vrows = actp.tile([128, HPC * HRST], BF, tag="vrows")
            # warmup source first in the Pool queue: the PE ramp can begin
            # ~0.5us in, before any DMA lands
            wsrc = actp.tile([128, 512], BF, tag="wsrc")
            nc.gpsimd.memset(wsrc[:], 0.0)
            # dummy sqrt pins the act table that holds Copy+Sqrt, so the
            # c-path later needs no table reload
            sqtrash = actp.tile([1, 1], F32, tag="sqtrash")
            nc.scalar.activation(sqtrash[:], wsrc[0:1, 0:1], AF.Sqrt)
            # aug columns (64 mod RST) must be ones; data cols are
            # overwritten by the v / k-transpose copies later
            nc.gpsimd.memset(vrows[:], 1.0)
            nc.gpsimd.memset(krows[:], 1.0)
            # per-head c rows for the rank-1 score aug term (both at
            # partition 0; scaled in place once the norms resolve)
            crow = [
                actp.tile([1, 128], BF, tag=f"crow{h}", name=f"crow{h}") for h in range(HPC)
            ]
            for h in range(HPC):
                nc.gpsimd.memset(crow[h][:], 1.0)

            def qcol(h, c0, c1):  # stacked q slice for head h
                return qks[h * 64 : (h + 1) * 64, c0:c1]

            def kcol(h, c0, c1):
                return qks[h * 64 : (h + 1) * 64, N + c0 : N + c1]

            sqp = {}

            # ================= warmup + phase A: q,k projections =================
            with tc.tile_pool(name="warm", bufs=1, space="PSUM") as warmp:
                wps = warmp.tile([128, 512], F32)
                for i in range(NWARM):
                    nc.tensor.matmul(wps[:], wsrc[:, 0:128], wsrc[:], start=True, stop=True)

            with tc.tile_pool(name="projps", bufs=1, space="PSUM") as pps:
                pq = [pps.tile([128, 512], F32, tag=f"pq{n0}", name=f"pq{n0}") for n0 in range(4)]
                pk = [pps.tile([128, 512], F32, tag=f"pk{n0}", name=f"pk{n0}") for n0 in range(3)]

                def drain(p, name, isq, n0):
                    # single stacked copy: the psum bank frees after one op
                    cs0 = (0 if isq else N) + n0 * 512
                    dst = qks[:, cs0 : cs0 + 512]
                    if drain.cnt % 2 == 0:
                        nc.vector.tensor_copy(dst, p[:])
                    else:
                        nc.scalar.copy(dst, p[:])
                    drain.cnt += 1
                    # squares for the norm stats, off the SBUF copy
                    sq = actp.tile([128, 512], BF, tag=f"sq{name}", name=f"sq{name}")
                    if drain.cnt % 2 == 0:
                        nc.gpsimd.tensor_mul(sq[:], dst, dst)
                    else:
                        nc.vector.tensor_mul(sq[:], dst, dst)
                    sqp[name] = sq

                drain.cnt = 0

                for k in range(KT):
                    ws = wqk_sb[:, k * 128 : (k + 1) * 128]
                    wsk = wqk_sb[:, D_MODEL + k * 128 : D_MODEL + (k + 1) * 128]
                    for n0 in range(4):
                        nc.tensor.matmul(
                            pq[n0][:],
                            ws,
                            xts[k][:, n0 * 512 : (n0 + 1) * 512],
                            start=(k == 0),
                            stop=(k == KT - 1),
                        )
                    if k == KT - 1:
                        for n0 in range(4):
                            drain(pq[n0], f"q{n0}", True, n0)
                    for n0 in range(3):
                        nc.tensor.matmul(
                            pk[n0][:],
                            wsk,
                            xts[k][:, n0 * 512 : (n0 + 1) * 512],
                            start=(k == 0),
                            stop=(k == KT - 1),
                        )
                    if k == KT - 1:
                        for n0 in range(3):
                            drain(pk[n0], f"k{n0}", False, n0)

                # deferred k n0=3 rotates into the freed warmup bank
                pk3 = pps.tile([128, 512], F32, tag="pq0", name="pk3")
                for k in range(KT):
                    nc.tensor.matmul(
                        pk3[:],
                        wqk_sb[:, D_MODEL + k * 128 : D_MODEL + (k + 1) * 128],
                        xts[k][:, 3 * 512 : 4 * 512],
                        start=(k == 0),
                        stop=(k == KT - 1),
                    )
                drain(pk3, "k3", False, 3)

            # ============ region 2: norms, v, k-transposes, attention ============
            with (
                tc.tile_pool(name="vkps", bufs=2, space="PSUM") as vkps,
                tc.tile_pool(name="bigps", bufs=2, space="PSUM") as bigps,
            ):
                nr16 = nf.tile([16, 1], BF)

                s_chain = {}
                s_snap = {}  # (h, ci) -> ([64,65] at base h*64, [1,65] aug row)
                oops_h = {}

                def vktr_chunk(ci):
                    ts0 = ci * 128
                    pv = vkps.tile([128, 128], F32, tag="vk", name="pv")
                    for k in range(KT):
                        nc.tensor.matmul(
                            pv[:],
                            xts[k][:, ts0 : ts0 + 128],
                            wvo_sb[:, k * 128 : (k + 1) * 128],
                            start=(k == 0),
                            stop=(k == KT - 1),
                        )
                    dst = ap3(vrows[:], ci * RST, [[HRST, HPC], [1, 64]])
                    src = ap3(pv[:], 0, [[64, HPC], [1, 64]])
                    nc.scalar.copy(dst, src)
                    ktp = vkps.tile([128, 128], BF, tag="vk", name="ktp")
                    nc.tensor.transpose(
                        ktp[:], qks[:, N + ts0 : N + ts0 + 128], ident
                    )
                    dst = ap3(krows[:], ci * RST, [[HRST, HPC], [1, 64]])
                    src = ap3(ktp[:], 0, [[64, HPC], [1, 64]])
                    nc.scalar.copy(dst, src)

                def sweep_chunk(ci):
                    dl = vkps.tile([65, 2 * 65], F32, tag="vk", name="dl")
                    for h in range(HPC):
                        nc.tensor.matmul(
                            dl[:, h * 65 : (h + 1) * 65],
                            krows[:, h * HRST + ci * RST : h * HRST + ci * RST + 65],
                            vrows[:, h * HRST + ci * RST : h * HRST + ci * RST + 65],
                            start=True,
                            stop=True,
                            skip_group_check=True,
                        )
                    ch = ssbp.tile(
                        [65, 2 * 65], F32, tag=f"sch{ci}", name=f"sch{ci}", bufs=1
                    )
                    if ci == 0:
                        nc.vector.tensor_copy(ch[:], dl[:])
                    else:
                        nc.vector.tensor_add(ch[:], dl[:], s_chain[ci - 1][:])
                    s_chain[ci] = ch
                    for h in range(HPC):
                        sm = ssbp.tile(
                            [128, 65], BF, tag=f"ssb{h}_{ci}", name=f"ssb{h}_{ci}", bufs=1
                        )
                        nc.gpsimd.tensor_copy(
                            sm[h * 64 : (h + 1) * 64, :], ch[0:64, h * 65 : (h + 1) * 65]
                        )
                        sa = ssbp.tile(
                            [1, 65], BF, tag=f"ssa{h}_{ci}", name=f"ssa{h}_{ci}", bufs=1
                        )
                        nc.gpsimd.tensor_copy(sa[:], ch[64:65, h * 65 : (h + 1) * 65])
                        s_snap[(h, ci)] = (sm, sa)

                def nrm_mms(nrm16):
                    for j in range(8):
                        name = f"q{j}" if j < 4 else f"k{j - 4}"
                        nc.tensor.matmul(
                            nrm16[:],
                            consts[:, 640 + 16 * j : 656 + 16 * j],
                            sqp[name][:],
                            start=(j == 0),
                            stop=(j == 7),
                            skip_group_check=True,
                        )

                def c_finalize(tr16):
                    # tr16 column j holds mm j's (h0,h1) maxima pair; q cols
                    # {h,2+h,..}, k cols {8+h,..}. tiny strided reduces -> c_h.
                    for h in range(HPC):
                        mqh = nf.tile([1, 1], F32, tag=f"mq{h}", name=f"mq{h}")
                        mkh = nf.tile([1, 1], F32, tag=f"mk{h}", name=f"mk{h}")
                        nc.vector.tensor_reduce(
                            mqh[:], ap3(tr16[:], h, [[2, 4]], nparts=1), AX.X, AluOpType.max
                        )
                        nc.vector.tensor_reduce(
                            mkh[:], ap3(tr16[:], 8 + h, [[2, 4]], nparts=1), AX.X, AluOpType.max
                        )
                        pr = nf.tile([1, 1], F32, tag=f"pr{h}", name=f"pr{h}")
                        nc.vector.tensor_mul(pr[:], mqh[:], mkh[:])
                        rt = nf.tile([1, 1], F32, tag=f"rt{h}", name=f"rt{h}")
                        nc.scalar.activation(rt[:], pr[:], AF.Sqrt)
                        nc.vector.tensor_scalar_mul(crow[h][:], crow[h][:], rt[:])

                def scores(sp):
                    ptj = bigps.tile([128, 2 * SPAN], F32, tag="big", name="ptj")
                    for h in range(HPC):
                        for i, ci in enumerate((2 * sp, 2 * sp + 1)):
                            cs = ptj[:, (2 * h + i) * CH : (2 * h + i + 1) * CH]
                            nc.tensor.matmul(
                                cs,
                                kcol(h, ci * CH, (ci + 1) * CH),
                                qcol(h, ci * CH, (ci + 1) * CH),
                                start=True,
                                stop=False,
                                skip_group_check=True,
                            )
                            nc.tensor.matmul(
                                cs,
                                crow[h][:],
                                onesall[0:1, :],
                                start=False,
                                stop=True,
                                skip_group_check=True,
                            )
                    mtd = mtp.tile([128, 2 * SPAN], BF, tag="mtd", name="mtd")
                    nc.vector.tensor_mul(mtd[:], ptj[:], dmask4)
                    return mtd

                def o_part(sp, mtd):
                    ca, cb_ = 2 * sp, 2 * sp + 1
                    vhrs = {
                        ca: vhrp.tile([128, 128], BF, tag="vhr", name="vhra"),
                        cb_: vhrp.tile([128, 128], BF, tag="vhr", name="vhrb"),
                    }
                    for h in range(HPC):
                        for i, ci in enumerate((ca, cb_)):
                            o = oops_h["p"].tile([128, 65], F32, tag="oop", name="o")
                            vr = vrows[:, h * HRST + ci * RST : h * HRST + ci * RST + 65]
                            nc.tensor.matmul(
                                o[:],
                                mtd[:, (2 * h + i) * CH : (2 * h + i + 1) * CH],
                                vr,
                                start=True,
                                stop=(ci == 0),
                            )
                            if ci > 0:
                                sm, sa = s_snap[(h, ci - 1)]
                                nc.tensor.matmul(
                                    o[:],
                                    qcol(h, ci * CH, (ci + 1) * CH),
                                    sm[h * 64 : (h + 1) * 64, :],
                                    start=False,
                                    stop=False,
                                )
                                nc.tensor.matmul(
                                    o[:],
                                    crow[h][:],
                                    sa[:],
                                    start=False,
                                    stop=True,
                                )
                            rec = recp.tile([128, 1], F32, tag="rec", name="rec")
                            nc.vector.reciprocal(rec[:], o[:, 64:65])
                            dst = vhrs[ci][:, h * 64 : (h + 1) * 64]
                            if (h + i) % 2 == 0:
                                nc.vector.tensor_scalar_mul(dst, o[:, 0:64], rec[:])
                            else:
                                nc.scalar.activation(dst, o[:, 0:64], AF.Copy, scale=rec[:])
                    return vhrs

                def vht_finish(sp, vhrs):
                    vtp = bigps.tile([128, SPAN], BF, tag="big", name="vtp")
                    for i, ci in enumerate((2 * sp, 2 * sp + 1)):
                        nc.tensor.transpose(vtp[:, i * CH : (i + 1) * CH], vhrs[ci][:], ident)
                        dst = vht[:, ci * CH : (ci + 1) * CH]
                        if i == 0:
                            nc.scalar.copy(dst, vtp[:, i * CH : (i + 1) * CH])
                        else:
                            nc.vector.tensor_copy(dst, vtp[:, i * CH : (i + 1) * CH])

                def outproj_row(r):
                    if True:
                        rs_ = slice(r * CH, (r + 1) * CH)
                        ob = osbp.tile([128, D_MODEL], BF, tag="osb", name="osb")
                        for n2 in range(D_MODEL // 512):
                            ns = slice(n2 * 512, (n2 + 1) * 512)
                            op = oops_h["p"].tile([128, 512], F32, tag="oop", name="opps")
                            nc.tensor.matmul(
                                op[:],
                                vht[:, rs_],
                                wvo_sb[:, D_MODEL + ns.start : D_MODEL + ns.stop],
                                start=True,
                                stop=True,
                            )
                            if (r + n2) % 2 == 0:
                                nc.vector.tensor_copy(ob[:, ns], op[:])
                            else:
                                nc.scalar.copy(ob[:, ns], op[:])
                        nc.sync.dma_start(out_d[rs_, :], ob[:])

                nxt = [0]

                def emit_chunks_until(limit):
                    while nxt[0] <= min(limit, NCH - 1):
                        ci = nxt[0]
                        vktr_chunk(ci)
                        if ci < NCH - 1:
                            sweep_chunk(ci)
                        nxt[0] += 1

                # prelude: chunks 0..3 cover the PE while the norm chain and
                # c resolve on the side engines; nrmps lives only here so its
                # bank can go to the 3-deep O/outproj pool afterwards
                with tc.tile_pool(name="nrmps", bufs=1, space="PSUM") as nps:
                    nrm16 = nps.tile([16, 512], F32, tag="nrm", name="nrm16")
                    tr16 = nps.tile([1, 16], BF, tag="nrm", name="tr16")
                    emit_chunks_until(3)
                    nrm_mms(nrm16)
                    nc.vector.tensor_reduce(nr16[:], nrm16[:], AX.X, AluOpType.max)
                    emit_chunks_until(5)
                    nc.tensor.transpose(tr16[:], nr16[:], ident[0:16, 0:16])
                    c_finalize(tr16)

                with tc.tile_pool(name="oops", bufs=4, space="PSUM") as oops_pool:
                    oops_h["p"] = oops_pool
                    vhr_prev = None
                    for sp in range(NSPAN):
                        emit_chunks_until(2 * sp + 2)
                        if sp >= 1:
                            vht_finish(sp - 1, vhr_prev)
                        emit_chunks_until(2 * sp + 3)
                        mtd = scores(sp)
                        if sp >= 1:
                            outproj_row(2 * sp - 2)
                            outproj_row(2 * sp - 1)
                        vhr_prev = o_part(sp, mtd)
                    vht_finish(NSPAN - 1, vhr_prev)
                    outproj_row(NCH - 2)
                    outproj_row(NCH - 1)

    nc.compile()
    return nc


def _consts():
    import ml_dtypes

    bf = ml_dtypes.bfloat16
    consts = np.zeros((128, 896), dtype=np.float32)
    consts[:, 0:128] = np.eye(128)
    j = np.arange(128)[:, None]
    i = np.arange(CH)[None, :]
    tri = (j <= i).astype(np.float32)
    for b in range(4):
        consts[:, 128 + b * CH : 128 + (b + 1) * CH] = tri
    # hindt16 blocks: mm j's stationary [128,16] has only cols 2j (head0
    # rows) and 2j+1 (head1 rows) set, so 8 accumulating matmuls stack
    # per-(proj,n0) norm rows into one [16,512] psum.
    for jj in range(8):
        for h in range(HPC):
            consts[h * 64 : (h + 1) * 64, 640 + 16 * jj + 2 * jj + h] = 1.0
    consts[:, 768:896] = 1.0  # onesall
    return consts.astype(bf)


def _in_maps(inputs):
    import ml_dtypes

    bf = ml_dtypes.bfloat16
    X = np.ascontiguousarray(np.asarray(inputs["X"], dtype=np.float32))
    xt = np.ascontiguousarray(X[0].T).astype(bf)  # [D_MODEL, N]
    wqt = np.ascontiguousarray(np.asarray(inputs["Wq"], np.float32).T).astype(bf)
    wkt = np.ascontiguousarray(np.asarray(inputs["Wk"], np.float32).T).astype(bf)
    wvt = np.ascontiguousarray(np.asarray(inputs["Wv"], np.float32).T).astype(bf)
    wot = np.ascontiguousarray(np.asarray(inputs["Wo"], np.float32).T).astype(bf)
    consts = _consts()

    def sb_layout(w):  # [1024, 128] -> [128, 8*128] (dm-chunk on partitions)
        return np.ascontiguousarray(
            w.reshape(KT, 128, DPC).transpose(1, 0, 2).reshape(128, KT * DPC)
        )

    in_maps = []
    for c in range(NCORES):
        cs = slice(c * DPC, (c + 1) * DPC)
        wqk = np.concatenate([sb_layout(wqt[:, cs]), sb_layout(wkt[:, cs])], axis=1)
        wvo = np.concatenate(
            [sb_layout(wvt[:, cs]), np.ascontiguousarray(wot[cs, :])], axis=1
        )
        in_maps.append(
            {
                "xt": xt,
                "wqk": np.ascontiguousarray(wqk),
                "wvo": np.ascontiguousarray(wvo),
                "consts": consts,
            }
        )
    return in_maps


def _run(inputs, trace=False):
    from concourse.bass_utils import run_bass_kernel_spmd

    if "nc" not in _CACHE:
        _CACHE["nc"] = _build()
    nc = _CACHE["nc"]
    in_maps = _in_maps(inputs)
    res = run_bass_kernel_spmd(nc, in_maps, core_ids=list(range(NCORES)), trace=trace)
    bo = np.asarray(inputs["bo"], dtype=np.float32)
    acc = np.zeros((N, D_MODEL), dtype=np.float32)
    for c in range(NCORES):
        acc += res.results[c]["out"].astype(np.float32)
    acc += bo[None, :]
    return acc.reshape(B, N, D_MODEL), res.exec_time_ns


def kernel(**inputs) -> np.ndarray:
    out, _ = _run(inputs, trace=False)
    return out


# revision 49
# speedup vs baseline: 1.0082x; 1.0023x over previous
"""Fastmax (p=1 causal linear attention) Trainium2 kernel, 8-core SPMD.

Sharding: data-parallel over heads (16 heads -> 2 per core). Each core
computes q/k/v projections for its 2 heads, the chunked causal linear
attention (a [65,65] prefix state per head carries S, ksum, vsum, count),
and a partial output projection. Host sums the 8 partial outputs + bias.

v3 design notes (cost-model driven):
  - xt streamed as big [128, N] DMAs; phase A (q,k) accumulates k-outer
    into 8 resident psum banks so PE consumption paces DMA arrival. PE
    warmup matmuls off a memset tile beat the p-state ramp.
  - q/k stay HEAD-STACKED [128, 2N] in SBUF: one [128,512] drain copy
    per projection psum, so psum banks recycle ~650ns after stop and
    the drain never serializes pass 2.
  - normalization: score' = <q,k> + qn*kn is a uniform 1/s scaling of
    the reference score, so num/den is invariant; the +c term enters
    scores via a rank-1 matmul with a tiny c-filled row, and enters
    the O accumulation via the state's aug row.
  - norm stats: squares of the stacked copies (Pool/DVE), 8 stationary-
    one-hot matmuls stack into one [16,512] psum, single DVE max-reduce,
    PE transpose, tiny strided reduces.
  - krows (token-major k for the state sweep) come from PE transposes
    of the stacked k block; aug columns are memset ones.
  - causal state is chunk-granular (128 tokens): per chunk
    O = masked-diag + q @ S_prev + 1 * S_prev[aug], no unmasked block.
"""

import sys

sys.path.insert(0, "/opt/trn_rl_repo")

import numpy as np

B, N, D_MODEL, H, D_HEAD = 1, 2048, 1024, 16, 64
NCORES = 8
HPC = H // NCORES  # heads per core
DPC = HPC * D_HEAD  # out dims per core (128)
CH = 128  # chunk (tokens)
SPAN = 256  # query span (2 chunks)
NSPAN = N // SPAN
KT = D_MODEL // 128  # contraction tiles for projections
NCH = N // CH  # token chunks (16)
RST = 80  # row-buffer stride per chunk (64 data + ones col + pad)
HRST = NCH * RST  # per-head stride inside krows/vrows
NWARM = 6  # PE warmup matmuls during initial DMA wait

_CACHE = {}


def _build():
    import concourse.bass as bass
    import concourse.tile as tile
    import concourse.mybir as mybir
    from concourse import bacc
    from concourse.alu_op_type import AluOpType

    BF = mybir.dt.bfloat16
    F32 = mybir.dt.float32
    AF = mybir.ActivationFunctionType
    AX = mybir.AxisListType

    nc = bacc.Bacc("TRN2", target_bir_lowering=False, debug=False, num_devices=NCORES)

    xt_d = nc.declare_dram_parameter("xt", [D_MODEL, N], BF, isOutput=False)
    wqk_d = nc.declare_dram_parameter("wqk", [128, 2 * D_MODEL], BF, isOutput=False)
    wvo_d = nc.declare_dram_parameter("wvo", [128, 2 * D_MODEL], BF, isOutput=False)
    consts_d = nc.declare_dram_parameter("consts", [128, 896], BF, isOutput=False)
    out_d = nc.declare_dram_parameter("out", [N, D_MODEL], BF, isOutput=True)

    def ap3(t, off, free_dims, nparts=128):
        # raw AP: partition dim step is the tile's row pitch (elements)
        pitch = t.ap[0][0]
        return bass.AP(t.tensor, t.offset + off, [[pitch, nparts]] + free_dims)

    with tile.TileContext(nc) as tc:
        with (
            tc.tile_pool(name="const", bufs=1) as constp,
            tc.tile_pool(name="wts", bufs=1) as wp,
            tc.tile_pool(name="acts", bufs=1) as actp,
            tc.tile_pool(name="mt", bufs=4) as mtp,
            tc.tile_pool(name="ssb", bufs=1) as ssbp,
            tc.tile_pool(name="recp", bufs=8) as recp,
            tc.tile_pool(name="vhrp", bufs=4) as vhrp,
            tc.tile_pool(name="osb", bufs=3) as osbp,
            tc.tile_pool(name="nf", bufs=1) as nf,
        ):
            consts = constp.tile([128, 896], BF)
            ident = consts[:, 0:128]
            dmask4 = consts[:, 128:640]
            onesall = consts[:, 768:896]

            # head-stacked q|k activations: rows h*64..h*64+63 = head h dims,
            # columns [q (N) | k (N)]
            qks = actp.tile([128, HPC * N], BF, tag="qks")

            wqk_sb = wp.tile([128, 2 * D_MODEL], BF)
            nc.sync.dma_start(wqk_sb[:, 0:D_MODEL], wqk_d[:, 0:D_MODEL])
            nc.sync.dma_start(wqk_sb[:, D_MODEL : 2 * D_MODEL], wqk_d[:, D_MODEL : 2 * D_MODEL])

            xts = []
            for k in range(KT):
                xtile = actp.tile([128, N], BF, tag=f"xt{k}", name=f"xt{k}")
                if k < 2:  # halves: smoother early PE pacing
                    nc.sync.dma_start(xtile[:, 0 : N // 2], xt_d[k * 128 : (k + 1) * 128, 0 : N // 2])
                    nc.sync.dma_start(xtile[:, N // 2 : N], xt_d[k * 128 : (k + 1) * 128, N // 2 : N])
                else:
                    nc.sync.dma_start(xtile[:], xt_d[k * 128 : (k + 1) * 128, :])
                if k == 1:  # consts not needed before pass 2
                    nc.sync.dma_start(consts[:], consts_d[:])
                xts.append(xtile)

            wvo_sb = wp.tile([128, 2 * D_MODEL], BF)
            nc.sync.dma_start(wvo_sb[:], wvo_d[:])

            vht = actp.tile([128, N], BF, tag="vht")
            krows = actp.tile([128, HPC * HRST], BF, tag="krows")
            vrows = actp.tile([128, HPC * HRST], BF, tag="vrows")
            # warmup source first in the Pool queue: the PE ramp can begin
            # ~0.5us in, before any DMA lands
            wsrc = actp.tile([128, 512], BF, tag="wsrc")
            nc.gpsimd.memset(wsrc[:], 0.0)
            # dummy sqrt pins the act table that holds Copy+Sqrt, so the
            # c-path later needs no table reload
            sqtrash = actp.tile([1, 1], F32, tag="sqtrash")
            nc.scalar.activation(sqtrash[:], wsrc[0:1, 0:1], AF.Sqrt)
            # aug columns (64 mod RST) must be ones; data cols are
            # overwritten by the v / k-transpose copies later
            nc.gpsimd.memset(vrows[:], 1.0)
            nc.gpsimd.memset(krows[:], 1.0)
            # per-head c rows for the rank-1 score aug term (both at
            # partition 0; scaled in place once the norms resolve)
            crow = [
                actp.tile([1, 128], BF, tag=f"crow{h}", name=f"crow{h}") for h in range(HPC)
            ]
            for h in range(HPC):
                nc.gpsimd.memset(crow[h][:], 1.0)

            def qcol(h, c0, c1):  # stacked q slice for head h
                return qks[h * 64 : (h + 1) * 64, c0:c1]

            def kcol(h, c0, c1):
                return qks[h * 64 : (h + 1) * 64, N + c0 : N + c1]

            sqp = {}

            # ================= warmup + phase A: q,k projections =================
            with tc.tile_pool(name="warm", bufs=1, space="PSUM") as warmp:
                wps = warmp.tile([128, 512], F32)
                for i in range(NWARM):
                    nc.tensor.matmul(wps[:], wsrc[:, 0:128], wsrc[:], start=True, stop=True)

            with tc.tile_pool(name="projps", bufs=1, space="PSUM") as pps:
                pq = [pps.tile([128, 512], F32, tag=f"pq{n0}", name=f"pq{n0}") for n0 in range(4)]
                pk = [pps.tile([128, 512], F32, tag=f"pk{n0}", name=f"pk{n0}") for n0 in range(3)]

                def drain(p, name, isq, n0):
                    # single stacked copy: the psum bank frees after one op
                    cs0 = (0 if isq else N) + n0 * 512
                    dst = qks[:, cs0 : cs0 + 512]
                    if drain.cnt % 2 == 0:
                        nc.vector.tensor_copy(dst, p[:])
                    else:
                        nc.scalar.copy(dst, p[:])
                    drain.cnt += 1
                    # squares for the norm stats, off the SBUF copy
                    sq = actp.tile([128, 512], BF, tag=f"sq{name}", name=f"sq{name}")
                    if drain.cnt % 2 == 0:
                        nc.gpsimd.tensor_mul(sq[:], dst, dst)
                    else:
                        nc.vector.tensor_mul(sq[:], dst, dst)
                    sqp[name] = sq

                drain.cnt = 0

                for k in range(KT):
                    ws = wqk_sb[:, k * 128 : (k + 1) * 128]
                    wsk = wqk_sb[:, D_MODEL + k * 128 : D_MODEL + (k + 1) * 128]
                    for n0 in range(4):
                        nc.tensor.matmul(
                            pq[n0][:],
                            ws,
                            xts[k][:, n0 * 512 : (n0 + 1) * 512],
                            start=(k == 0),
                            stop=(k == KT - 1),
                        )
                    if k == KT - 1:
                        for n0 in range(4):
                            drain(pq[n0], f"q{n0}", True, n0)
                    for n0 in range(3):
                        nc.tensor.matmul(
                            pk[n0][:],
                            wsk,
                            xts[k][:, n0 * 512 : (n0 + 1) * 512],
                            start=(k == 0),
                            stop=(k == KT - 1),
                        )
                    if k == KT - 1:
                        for n0 in range(3):
                            drain(pk[n0], f"k{n0}", False, n0)

                # deferred k n0=3 rotates into the freed warmup bank
                pk3 = pps.tile([128, 512], F32, tag="pq0", name="pk3")
                for k in range(KT):
                    nc.tensor.matmul(
                        pk3[:],
                        wqk_sb[:, D_MODEL + k * 128 : D_MODEL + (k + 1) * 128],
                        xts[k][:, 3 * 512 : 4 * 512],
                        start=(k == 0),
                        stop=(k == KT - 1),
                    )
                drain(pk3, "k3", False, 3)

            # ============ region 2: norms, v, k-transposes, attention ============
            with (
                tc.tile_pool(name="vkps", bufs=2, space="PSUM") as vkps,
                tc.tile_pool(name="bigps", bufs=2, space="PSUM") as bigps,
            ):
                nr16 = nf.tile([16, 1], BF)

                s_chain = {}
                s_snap = {}  # (h, ci) -> ([64,65] at base h*64, [1,65] aug row)
                oops_h = {}

                def vktr_chunk(ci):
                    ts0 = ci * 128
                    pv = vkps.tile([128, 128], F32, tag="vk", name="pv")
                    for k in range(KT):
                        nc.tensor.matmul(
                            pv[:],
                            xts[k][:, ts0 : ts0 + 128],
                            wvo_sb[:, k * 128 : (k + 1) * 128],
                            start=(k == 0),
                            stop=(k == KT - 1),
                        )
                    dst = ap3(vrows[:], ci * RST, [[HRST, HPC], [1, 64]])
                    src = ap3(pv[:], 0, [[64, HPC], [1, 64]])
                    if ci % 2 == 0:
                        nc.scalar.copy(dst, src)
                    else:
                        nc.vector.tensor_copy(dst, src)
                    ktp = vkps.tile([128, 128], BF, tag="vk", name="ktp")
                    nc.tensor.transpose(
                        ktp[:], qks[:, N + ts0 : N + ts0 + 128], ident
                    )
                    dst = ap3(krows[:], ci * RST, [[HRST, HPC], [1, 64]])
                    src = ap3(ktp[:], 0, [[64, HPC], [1, 64]])
                    if ci % 2 == 0:
                        nc.vector.tensor_copy(dst, src)
                    else:
                        nc.scalar.copy(dst, src)

                def sweep_chunk(ci):
                    dl = vkps.tile([65, 2 * 65], F32, tag="vk", name="dl")
                    for h in range(HPC):
                        nc.tensor.matmul(
                            dl[:, h * 65 : (h + 1) * 65],
                            krows[:, h * HRST + ci * RST : h * HRST + ci * RST + 65],
                            vrows[:, h * HRST + ci * RST : h * HRST + ci * RST + 65],
                            start=True,
                            stop=True,
                            skip_group_check=True,
                        )
                    ch = ssbp.tile(
                        [65, 2 * 65], F32, tag=f"sch{ci}", name=f"sch{ci}", bufs=1
                    )
                    if ci == 0:
                        nc.vector.tensor_copy(ch[:], dl[:])
                    else:
                        nc.vector.tensor_add(ch[:], dl[:], s_chain[ci - 1][:])
                    s_chain[ci] = ch
                    for h in range(HPC):
                        sm = ssbp.tile(
                            [128, 65], BF, tag=f"ssb{h}_{ci}", name=f"ssb{h}_{ci}", bufs=1
                        )
                        nc.gpsimd.tensor_copy(
                            sm[h * 64 : (h + 1) * 64, :], ch[0:64, h * 65 : (h + 1) * 65]
                        )
                        sa = ssbp.tile(
                            [1, 65], BF, tag=f"ssa{h}_{ci}", name=f"ssa{h}_{ci}", bufs=1
                        )
                        nc.gpsimd.tensor_copy(sa[:], ch[64:65, h * 65 : (h + 1) * 65])
                        s_snap[(h, ci)] = (sm, sa)

                def nrm_mms(nrm16):
                    for j in range(8):
                        name = f"q{j}" if j < 4 else f"k{j - 4}"
                        nc.tensor.matmul(
                            nrm16[:],
                            consts[:, 640 + 16 * j : 656 + 16 * j],
                            sqp[name][:],
                            start=(j == 0),
                            stop=(j == 7),
                            skip_group_check=True,
                        )

                def c_finalize(tr16):
                    # tr16 column j holds mm j's (h0,h1) maxima pair; q cols
                    # {h,2+h,..}, k cols {8+h,..}. tiny strided reduces -> c_h.
                    for h in range(HPC):
                        mqh = nf.tile([1, 1], F32, tag=f"mq{h}", name=f"mq{h}")
                        mkh = nf.tile([1, 1], F32, tag=f"mk{h}", name=f"mk{h}")
                        nc.vector.tensor_reduce(
                            mqh[:], ap3(tr16[:], h, [[2, 4]], nparts=1), AX.X, AluOpType.max
                        )
                        nc.vector.tensor_reduce(
                            mkh[:], ap3(tr16[:], 8 + h, [[2, 4]], nparts=1), AX.X, AluOpType.max
                        )
                        pr = nf.tile([1, 1], F32, tag=f"pr{h}", name=f"pr{h}")
                        nc.vector.tensor_mul(pr[:], mqh[:], mkh[:])
                        rt = nf.tile([1, 1], F32, tag=f"rt{h}", name=f"rt{h}")
                        nc.scalar.activation(rt[:], pr[:], AF.Sqrt)
                        nc.vector.tensor_scalar_mul(crow[h][:], crow[h][:], rt[:])

                def scores(sp):
                    ptj = bigps.tile([128, 2 * SPAN], F32, tag="big", name="ptj")
                    for h in range(HPC):
                        for i, ci in enumerate((2 * sp, 2 * sp + 1)):
                            cs = ptj[:, (2 * h + i) * CH : (2 * h + i + 1) * CH]
                            nc.tensor.matmul(
                                cs,
                                kcol(h, ci * CH, (ci + 1) * CH),
                                qcol(h, ci * CH, (ci + 1) * CH),
                                start=True,
                                stop=False,
                                skip_group_check=True,
                            )
                            nc.tensor.matmul(
                                cs,
                                crow[h][:],
                                onesall[0:1, :],
                                start=False,
                                stop=True,
                                skip_group_check=True,
                            )
                    mtd = mtp.tile([128, 2 * SPAN], BF, tag="mtd", name="mtd")
                    nc.vector.tensor_mul(mtd[:], ptj[:], dmask4)
                    return mtd

                def o_part(sp, mtd):
                    ca, cb_ = 2 * sp, 2 * sp + 1
                    vhrs = {
                        ca: vhrp.tile([128, 128], BF, tag="vhr", name="vhra"),
                        cb_: vhrp.tile([128, 128], BF, tag="vhr", name="vhrb"),
                    }
                    for h in range(HPC):
                        for i, ci in enumerate((ca, cb_)):
                            o = oops_h["p"].tile([128, 65], F32, tag="oop", name="o")
                            vr = vrows[:, h * HRST + ci * RST : h * HRST + ci * RST + 65]
                            nc.tensor.matmul(
                                o[:],
                                mtd[:, (2 * h + i) * CH : (2 * h + i + 1) * CH],
                                vr,
                                start=True,
                                stop=(ci == 0),
                            )
                            if ci > 0:
                                sm, sa = s_snap[(h, ci - 1)]
                                nc.tensor.matmul(
                                    o[:],
                                    qcol(h, ci * CH, (ci + 1) * CH),
                                    sm[h * 64 : (h + 1) * 64, :],
                                    start=False,
                                    stop=False,
                                )
                                nc.tensor.matmul(
                                    o[:],
                                    crow[h][:],
                                    sa[:],
                                    start=False,
                                    stop=True,
                                )
                            rec = recp.tile([128, 1], F32, tag="rec", name="rec")
                            nc.vector.reciprocal(rec[:], o[:, 64:65])
                            dst = vhrs[ci][:, h * 64 : (h + 1) * 64]
                            if (h + i) % 2 == 0:
                                nc.vector.tensor_scalar_mul(dst, o[:, 0:64], rec[:])
                            else:
                                nc.scalar.activation(dst, o[:, 0:64], AF.Copy, scale=rec[:])
                    return vhrs

                def vht_finish(sp, vhrs):
                    vtp = bigps.tile([128, SPAN], BF, tag="big", name="vtp")
                    for i, ci in enumerate((2 * sp, 2 * sp + 1)):
                        nc.tensor.transpose(vtp[:, i * CH : (i + 1) * CH], vhrs[ci][:], ident)
                        dst = vht[:, ci * CH : (ci + 1) * CH]
                        if i == 0:
                            nc.scalar.copy(dst, vtp[:, i * CH : (i + 1) * CH])
                        else:
                            nc.vector.tensor_copy(dst, vtp[:, i * CH : (i + 1) * CH])

                def outproj_row(r):
                    if True:
                        rs_ = slice(r * CH, (r + 1) * CH)
                        ob = osbp.tile([128, D_MODEL], BF, tag="osb", name="osb")
                        for n2 in range(D_MODEL // 512):
                            ns = slice(n2 * 512, (n2 + 1) * 512)
                            op = oops_h["p"].tile([128, 512], F32, tag="oop", name="opps")
                            nc.tensor.matmul(
                                op[:],
                                vht[:, rs_],
                                wvo_sb[:, D_MODEL + ns.start : D_MODEL + ns.stop],
                                start=True,
                                stop=True,
                            )
                            if (r + n2) % 2 == 0:
                                nc.vector.tensor_copy(ob[:, ns], op[:])
                            else:
                                nc.scalar.copy(ob[:, ns], op[:])
                        nc.sync.dma_start(out_d[rs_, :], ob[:])

                nxt = [0]

                def emit_chunks_until(limit):
                    while nxt[0] <= min(limit, NCH - 1):
                        ci = nxt[0]
                        vktr_chunk(ci)
                        if ci < NCH - 1:
                            sweep_chunk(ci)
                        nxt[0] += 1

                # prelude: chunks 0..3 cover the PE while the norm chain and
                # c resolve on the side engines; nrmps lives only here so its
                # bank can go to the 3-deep O/outproj pool afterwards
                with tc.tile_pool(name="nrmps", bufs=1, space="PSUM") as nps:
                    nrm16 = nps.tile([16, 512], F32, tag="nrm", name="nrm16")
                    tr16 = nps.tile([1, 16], BF, tag="nrm", name="tr16")
                    emit_chunks_until(3)
                    nrm_mms(nrm16)
                    nc.vector.tensor_reduce(nr16[:], nrm16[:], AX.X, AluOpType.max)
                    emit_chunks_until(5)
                    nc.tensor.transpose(tr16[:], nr16[:], ident[0:16, 0:16])
                    c_finalize(tr16)

                with tc.tile_pool(name="oops", bufs=4, space="PSUM") as oops_pool:
                    oops_h["p"] = oops_pool
                    vhr_prev = None
                    for sp in range(NSPAN):
                        emit_chunks_until(2 * sp + 2)
                        if sp >= 1:
                            vht_finish(sp - 1, vhr_prev)
                        emit_chunks_until(2 * sp + 3)
                        mtd = scores(sp)
                        if sp >= 1:
                            outproj_row(2 * sp - 2)
                            outproj_row(2 * sp - 1)
                        vhr_prev = o_part(sp, mtd)
                    vht_finish(NSPAN - 1, vhr_prev)
                    outproj_row(NCH - 2)
                    outproj_row(NCH - 1)

    nc.compile()
    return nc


def _consts():
    import ml_dtypes

    bf = ml_dtypes.bfloat16
    consts = np.zeros((128, 896), dtype=np.float32)
    consts[:, 0:128] = np.eye(128)
    j = np.arange(128)[:, None]
    i = np.arange(CH)[None, :]
    tri = (j <= i).astype(np.float32)
    for b in range(4):
        consts[:, 128 + b * CH : 128 + (b + 1) * CH] = tri
    # hindt16 blocks: mm j's stationary [128,16] has only cols 2j (head0
    # rows) and 2j+1 (head1 rows) set, so 8 accumulating matmuls stack
    # per-(proj,n0) norm rows into one [16,512] psum.
    for jj in range(8):
        for h in range(HPC):
            consts[h * 64 : (h + 1) * 64, 640 + 16 * jj + 2 * jj + h] = 1.0
    consts[:, 768:896] = 1.0  # onesall
    return consts.astype(bf)


def _in_maps(inputs):
    import ml_dtypes

    bf = ml_dtypes.bfloat16
    X = np.ascontiguousarray(np.asarray(inputs["X"], dtype=np.float32))
    xt = np.ascontiguousarray(X[0].T).astype(bf)  # [D_MODEL, N]
    wqt = np.ascontiguousarray(np.asarray(inputs["Wq"], np.float32).T).astype(bf)
    wkt = np.ascontiguousarray(np.asarray(inputs["Wk"], np.float32).T).astype(bf)
    wvt = np.ascontiguousarray(np.asarray(inputs["Wv"], np.float32).T).astype(bf)
    wot = np.ascontiguousarray(np.asarray(inputs["Wo"], np.float32).T).astype(bf)
    consts = _consts()

    def sb_layout(w):  # [1024, 128] -> [128, 8*128] (dm-chunk on partitions)
        return np.ascontiguousarray(
            w.reshape(KT, 128, DPC).transpose(1, 0, 2).reshape(128, KT * DPC)
        )

    in_maps = []
    for c in range(NCORES):
        cs = slice(c * DPC, (c + 1) * DPC)
        wqk = np.concatenate([sb_layout(wqt[:, cs]), sb_layout(wkt[:, cs])], axis=1)
        wvo = np.concatenate(
            [sb_layout(wvt[:, cs]), np.ascontiguousarray(wot[cs, :])], axis=1
        )
        in_maps.append(
            {
                "xt": xt,
                "wqk": np.ascontiguousarray(wqk),
                "wvo": np.ascontiguousarray(wvo),
                "consts": consts,
            }
        )
    return in_maps


def _run(inputs, trace=False):
    from concourse.bass_utils import run_bass_kernel_spmd

    if "nc" not in _CACHE:
        _CACHE["nc"] = _build()
    nc = _CACHE["nc"]
    in_maps = _in_maps(inputs)
    res = run_bass_kernel_spmd(nc, in_maps, core_ids=list(range(NCORES)), trace=trace)
    bo = np.asarray(inputs["bo"], dtype=np.float32)
    acc = np.zeros((N, D_MODEL), dtype=np.float32)
    for c in range(NCORES):
        acc += res.results[c]["out"].astype(np.float32)
    acc += bo[None, :]
    return acc.reshape(B, N, D_MODEL), res.exec_time_ns


def kernel(**inputs) -> np.ndarray:
    out, _ = _run(inputs, trace=False)
    return out
